# revision 22
# baseline (speedup 1.0000x reference)
"""Batched GAT (dense adjacency) Trainium2 Bass kernel.

Shards the batch (B=16) across 8 NeuronCores (2 samples/core), replicating
the small GAT weights. Per sample, on-device:
  h   = x @ W                      (PE, via PE-transposed x)
  e   = a_src/a_dst logit rows     (PE, small matmuls off h^T)
  p^T = mask * exp(prelu(e_dst[i] + e_src[j]))   (ACT Prelu+Exp, DVE mask)
  out = (p^T @ h) / rowsum + bias                (PE agg, bf16)

Attention is built TRANSPOSED ([j, i], source nodes on partitions) so both
the softmax denominators (ones-vector matmuls) and the aggregation are PE
matmuls; adj is PE-transposed on device; e_dst rows are broadcast across
partitions via a DRAM round-trip. Aggregation and denominators run in bf16
(numerator/denominator rounding errors largely cancel); the logit path is
f32 with f32r for the wide matmuls. Aggregation outputs are packed two
node-chunks per PSUM bank so sample k+1's preamble (emitted interleaved
with sample k's attention phase) always has free PSUM slots.
"""

import numpy as np

import concourse.bass as bass
import concourse.bacc as bacc
import concourse.tile as tile
from concourse import mybir
from concourse.bass_utils import run_bass_kernel_spmd
from concourse.masks import make_identity

F32 = mybir.dt.float32
F32R = mybir.dt.float32r
BF16 = mybir.dt.bfloat16
AF = mybir.ActivationFunctionType
ALU = mybir.AluOpType

P = 128          # partitions
N = 1024         # nodes
D = 256          # input feature dim
H = 4            # heads
F = 64           # per-head dim
HF = H * F       # 256
NCH = N // P     # 8 chunks of nodes
NCORES = 8
BPC = 2          # batch samples per core
NEG_SLOPE = 0.2


def build_nc(num_devices=NCORES, repeat=1):
    nc = bacc.Bacc("TRN2", target_bir_lowering=False, debug=False,
                   num_devices=num_devices)
    x_d = nc.dram_tensor("x", [BPC, N, D], F32, kind="ExternalInput")
    adj_d = nc.dram_tensor("adj", [BPC, N, N], F32, kind="ExternalInput")
    w_d = nc.dram_tensor("W", [D, HF], F32, kind="ExternalInput")
    acat_d = nc.dram_tensor("acat", [HF, 2 * H], F32, kind="ExternalInput")
    wa_d = nc.dram_tensor("wa", [D, 2 * H], F32, kind="ExternalInput")
    bias_d = nc.dram_tensor("bias", [HF], F32, kind="ExternalInput")
    out_d = nc.dram_tensor("out", [BPC, N, HF], F32, kind="ExternalOutput")

    with tile.TileContext(nc) as tc:
        with (
            tc.tile_pool(name="consts", bufs=1) as consts,
            tc.tile_pool(name="xs", bufs=1) as p_xs,
            tc.tile_pool(name="xt", bufs=2) as p_xt,
            tc.tile_pool(name="haug", bufs=2) as p_haug,
            tc.tile_pool(name="erow", bufs=2) as p_erow,
            tc.tile_pool(name="ecol", bufs=2) as p_ecol,
            tc.tile_pool(name="bd", bufs=2) as p_bd,
            tc.tile_pool(name="mask", bufs=2) as p_mask,
            tc.tile_pool(name="adj", bufs=2) as p_adj,
            tc.tile_pool(name="pt", bufs=6) as p_pt,
            tc.tile_pool(name="pm", bufs=6) as p_pm,
            tc.tile_pool(name="ssum", bufs=2) as p_ssum,
            tc.tile_pool(name="ot", bufs=4) as p_ot,
            tc.tile_pool(name="ps", bufs=8, space="PSUM") as p_ps,
            tc.tile_pool(name="dram", bufs=2, space="DRAM") as p_dram,
        ):
            ident = consts.tile([P, P], F32)
            make_identity(nc, ident)
            w_sb = consts.tile([P, 2, HF], F32)
            for dc in range(2):
                nc.sync.dma_start(w_sb[:, dc, :], w_d[dc * P:(dc + 1) * P, :])
            bias_bc = consts.tile([P, HF], F32)
            nc.sync.dma_start(bias_bc[:], bias_d[:].partition_broadcast(P))
            w_sbr = consts.tile([P, 2, HF], F32R)
            nc.vector.tensor_copy(w_sbr[:], w_sb[:])
            wa_sb = consts.tile([P, 2, 2 * H], F32)
            for dc in range(2):
                nc.sync.dma_start(wa_sb[:, dc, :], wa_d[dc * P:(dc + 1) * P, :])
            wa_sbr = consts.tile([P, 2, 2 * H], F32R)
            nc.vector.tensor_copy(wa_sbr[:], wa_sb[:])
            alpha_col = consts.tile([P, 1], F32)
            nc.vector.memset(alpha_col[:], NEG_SLOPE)
            ones_rep = consts.tile([P, 64], BF16)
            nc.vector.memset(ones_rep[:], 1.0)

            def phase_abm(b):
                """Generator: yields after each small chunk so the caller can
                interleave this sample's preamble into the previous sample's
                attention phase. Final yield carries the state tuple."""
                # ---- A: load x, PE-transpose to xT [d, i] ----
                xt_t = p_xt.tile([P, 2, N], F32R, tag="xt", name=f"xt{b}")
                xs_all = p_xs.tile([P, NCH, D], F32, tag="xs", name=f"xs{b}")
                nc.sync.dma_start(
                    xs_all[:], x_d[b].rearrange("(c p) d -> p c d", p=P))
                for icg in range(2):
                    for dc in range(2):
                        psx = p_ps.tile([P, 512], F32, tag="u",
                                        name=f"psx{b}_{icg}{dc}")
                        for ic4 in range(4):
                            ic = icg * 4 + ic4
                            nc.tensor.transpose(psx[:, ic4 * P:(ic4 + 1) * P],
                                                xs_all[:, ic, dc * P:(dc + 1) * P],
                                                ident[:])
                        nc.vector.tensor_copy(
                            xt_t[:, dc, icg * 512:(icg + 1) * 512], psx[:])
                        yield

                # E rows [2H, N] = (W @ acat)^T @ xT
                erow_t = p_erow.tile([P, N], F32, tag="erow", name=f"erow{b}")
                nc.gpsimd.memset(erow_t[:], 0.0)
                for nh in range(2):
                    pe_ = p_ps.tile([P, 512], F32, tag="u", name=f"pse{b}{nh}")
                    for dc in range(2):
                        nc.tensor.matmul(pe_[0:2 * H, :],
                                         wa_sbr[:, dc, :],
                                         xt_t[:, dc, nh * 512:(nh + 1) * 512],
                                         start=(dc == 0), stop=(dc == 1))
                    nc.vector.tensor_copy(erow_t[0:2 * H, nh * 512:(nh + 1) * 512],
                                          pe_[0:2 * H, :])
                    yield

                # e columns: transpose E rows -> [node_part, 2H] per chunk
                ecol_t = p_ecol.tile([P, NCH, 2 * H], F32, tag="ecol",
                                     name=f"ecol{b}")
                for jc2 in range(4):
                    pec = p_ps.tile([P, 2, P], F32, tag="u", name=f"pec{b}{jc2}")
                    for k in range(2):
                        jc = jc2 * 2 + k
                        nc.tensor.transpose(pec[:, k, :],
                                            erow_t[:, jc * P:(jc + 1) * P],
                                            ident[:])
                        nc.vector.tensor_copy(ecol_t[:, jc, :], pec[:, k, 0:2 * H])
                    yield

                # e_dst rows broadcast across partitions via DRAM round-trip
                scr = p_dram.tile([2 * H, N], F32, tag="scr", name=f"scr{b}")
                nc.sync.dma_start(scr[:], erow_t[0:2 * H, :])
                bd_t = p_bd.tile([P, H, N], F32, tag="bd", name=f"bd{b}")
                for h in range(H):
                    nc.sync.dma_start(bd_t[:, h, :],
                                      scr[2 * h + 1, :].partition_broadcast(P))
                yield

                # ---- h (bf16) for aggregation ----
                haug_t = p_haug.tile([P, NCH, H, F], BF16, tag="haug",
                                     name=f"haug{b}")
                for ic in range(NCH):
                    ph = p_ps.tile([P, HF], F32, tag="u", name=f"psh{b}{ic}")
                    for dc in range(2):
                        nc.tensor.matmul(ph[:],
                                         xt_t[:, dc, ic * P:(ic + 1) * P],
                                         w_sbr[:, dc, :],
                                         start=(dc == 0), stop=(dc == 1))
                    nc.vector.tensor_copy(
                        haug_t[:, ic, :, :],
                        ph.rearrange("p (h f) -> p h f", h=H))
                    if ic % 2 == 1:
                        yield

                # ---- M: transposed edge mask (adj^T > 0.5), bf16 ----
                mask_t = p_mask.tile([P, NCH, N], BF16, tag="mask",
                                     name=f"mask{b}")
                for jcp in range(4):   # pairs of j-chunks; 1 psum bank at a time
                    at_all = p_adj.tile([P, NCH, 2 * P], F32, tag="adj",
                                        name=f"at{b}{jcp}")
                    nc.sync.dma_start(
                        at_all[:],
                        adj_d[b].rearrange("(c p) j -> p c j", p=P)
                        [:, :, jcp * 2 * P:(jcp + 1) * 2 * P])
                    for k in range(2):      # the two j-chunks of this pair
                        jc = jcp * 2 + k
                        for g in range(2):  # source-chunk groups of 4
                            pmx = p_ps.tile([P, 512], F32, tag="u",
                                            name=f"pmx{b}_{jcp}{k}{g}")
                            for ib4 in range(4):
                                ib = g * 4 + ib4
                                nc.tensor.transpose(
                                    pmx[:, ib4 * P:(ib4 + 1) * P],
                                    at_all[:, ib, k * P:(k + 1) * P], ident[:])
                            nc.vector.tensor_scalar(
                                out=mask_t[:, jc, g * 512:(g + 1) * 512],
                                in0=pmx[:],
                                scalar1=0.5, scalar2=None, op0=ALU.is_gt)
                            yield

                yield (haug_t, ecol_t, bd_t, mask_t)

            def run_abm(b):
                """Run the full preamble for sample b, return state."""
                st = None
                for st in phase_abm(b):
                    pass
                return st

            def phase_de(b, state, interleave=None):
                """Attention + aggregation for sample b; optionally pull one
                chunk of `interleave` (next sample's preamble) per tile."""
                haug_t, ecol_t, bd_t, mask_t = state
                # aggregation outputs: 2 node-chunks packed per PSUM bank
                pouts = [p_ps.tile([P, 512], F32, tag="u", name=f"po{b}_{i}")
                         for i in range(NCH // 2)]
                # softmax denominators: one bank per i-half; head h occupies
                # rows [32h, 32h+32) (M-replicated rows). h3 is written first
                # as a [64, 512] block at base 64, then h0-2 overwrite 0..95.
                sums = [p_ps.tile([P, 512], F32, tag="u", name=f"sm{b}_{i}")
                        for i in range(2)]

                def sum_slot(h, half, for_write=False):
                    if for_write and h == 3:
                        return sums[half][64:128, :]
                    if for_write:
                        return sums[half][32 * h:32 * (h + 1), :]
                    return sums[half][32 * h:32 * h + 1, :]
                nxt = None
                for h in (3, 0, 1, 2):
                    for jc in range(NCH):
                        pt = p_pt.tile([P, N], F32, tag="pt", name=f"pt{b}{h}{jc}")
                        nc.scalar.activation(
                            out=pt[:], in_=bd_t[:, h, :], func=AF.Prelu,
                            bias=ecol_t[:, jc, 2 * h:2 * h + 1],
                            scale=1.0, alpha=alpha_col[:])
                        pe_b = p_pm.tile([P, N], BF16, tag="peb",
                                         name=f"pe{b}{h}{jc}")
                        nc.scalar.activation(out=pe_b[:], in_=pt[:], func=AF.Exp,
                                             bias=0.0, scale=1.0)
                        pm = p_pm.tile([P, N], BF16, tag="pm", name=f"pm{b}{h}{jc}")
                        eng = nc.gpsimd if jc % 3 == 2 else nc.vector
                        eng.tensor_tensor(out=pm[:], in0=pe_b[:],
                                          in1=mask_t[:, jc, :],
                                          op=ALU.mult)
                        first = (h == 3 and jc == 0)
                        last = (h == 2 and jc == NCH - 1)
                        for ic in range(NCH):
                            nc.tensor.matmul(
                                pouts[ic // 2][:, (ic % 2) * 256 + h * F:
                                               (ic % 2) * 256 + (h + 1) * F],
                                pm[:, ic * P:(ic + 1) * P],
                                haug_t[:, jc, h, :],
                                start=(first and ic % 2 == 0),
                                stop=(last and ic % 2 == 1))
                        nrep = 64 if h == 3 else 32
                        for half in range(2):
                            nc.tensor.matmul(
                                sum_slot(h, half, for_write=True),
                                ones_rep[:, 0:nrep],
                                pm[:, half * 512:(half + 1) * 512],
                                start=(jc == 0), stop=(jc == NCH - 1),
                                skip_group_check=(h != 3))
                        if interleave is not None:
                            nxt = next(interleave, nxt)

                # ---- denominators -> per-chunk reciprocal columns ----
                ssum_sb = p_ssum.tile([P, N], F32, tag="ssum", name=f"ss{b}")
                nc.gpsimd.memset(ssum_sb[:], 1.0)
                for half in range(2):
                    for h in range(H):
                        nc.vector.tensor_copy(
                            ssum_sb[32 * h:32 * h + 1,
                                    half * 512:(half + 1) * 512],
                            sum_slot(h, half))
                recip_t = p_ssum.tile([P, NCH, H], F32, tag="recip",
                                      name=f"rc{b}")
                for ic2 in range(4):
                    prc = p_ps.tile([P, 2, P], F32, tag="u", name=f"prc{b}{ic2}")
                    for k in range(2):
                        ic = ic2 * 2 + k
                        nc.tensor.transpose(prc[:, k, :],
                                            ssum_sb[:, ic * P:(ic + 1) * P],
                                            ident[:])
                        prcv = prc[:, k, :].rearrange("p (h c) -> p h c", c=32)
                        nc.vector.reciprocal(recip_t[:, ic, :], prcv[:, :, 0])

                # ---- normalize + bias + store ----
                for ic in range(NCH):
                    po = pouts[ic // 2].rearrange(
                        "p (q h f) -> p q h f", q=2, h=H)[:, ic % 2]
                    ot = p_ot.tile([P, HF], F32, tag="ot", name=f"ot{b}{ic}")
                    otv = ot.rearrange("p (h f) -> p h f", h=H)
                    rb = recip_t[:, ic, :].unsqueeze(2).broadcast_to([P, H, F])
                    nc.vector.tensor_tensor(out=otv[:], in0=po[:],
                                            in1=rb, op=ALU.mult)
                    nc.gpsimd.tensor_tensor(out=ot[:], in0=ot[:], in1=bias_bc[:],
                                            op=ALU.add)
                    nc.sync.dma_start(out_d[b, ic * P:(ic + 1) * P, :], ot[:])
                return nxt

            def body():
                st0 = run_abm(0)
                gen1 = phase_abm(1)
                st1 = phase_de(0, st0, interleave=gen1)
                for last in gen1:   # drain any remaining preamble chunks
                    st1 = last
                phase_de(1, st1)

            if repeat == 1:
                body()
            else:
                with tc.For_i(0, repeat, 1):
                    body()

    nc.compile()
    return nc


_NC_CACHE = {}


def _get_nc():
    if "nc" not in _NC_CACHE:
        _NC_CACHE["nc"] = build_nc()
    return _NC_CACHE["nc"]


def _prep_weights(W, a_src, a_dst, bias):
    W2 = np.ascontiguousarray(W.reshape(D, HF).astype(np.float32))
    acat = np.zeros((HF, 2 * H), np.float32)
    for h in range(H):
        acat[h * F:(h + 1) * F, 2 * h] = a_src[h]
        acat[h * F:(h + 1) * F, 2 * h + 1] = a_dst[h]
    wa = np.ascontiguousarray((W2 @ acat).astype(np.float32))
    return W2, acat, wa, np.ascontiguousarray(bias.astype(np.float32))


def kernel(x, adj, W, a_src, a_dst, bias):
    x = np.asarray(x, dtype=np.float32)
    adj = np.asarray(adj, dtype=np.float32)
    W2, acat, wa, biasv = _prep_weights(np.asarray(W), np.asarray(a_src),
                                        np.asarray(a_dst), np.asarray(bias))
    nc = _get_nc()
    in_maps = []
    for c in range(NCORES):
        in_maps.append({
            "x": np.ascontiguousarray(x[c * BPC:(c + 1) * BPC]),
            "adj": np.ascontiguousarray(adj[c * BPC:(c + 1) * BPC]),
            "W": W2, "acat": acat, "wa": wa, "bias": biasv,
        })
    r = run_bass_kernel_spmd(nc, in_maps, core_ids=list(range(NCORES)))
    return np.concatenate([r.results[c]["out"] for c in range(NCORES)], axis=0)


# revision 26
# speedup vs baseline: 1.4493x; 1.4493x over previous
"""Batched GAT (dense adjacency) Trainium2 Bass kernel.

Shards the batch (B=16) across 8 NeuronCores (2 samples/core), replicating
the small GAT weights. Per sample, on-device:
  h   = x @ W                      (PE, via PE-transposed x)
  e   = a_src/a_dst logit rows     (PE, small matmuls off h^T)
  p^T = mask * exp(prelu(e_dst[i] + e_src[j]))   (ACT Prelu+Exp, DVE mask)
  out = (p^T @ h) / rowsum + bias                (PE agg, bf16)

Attention is built TRANSPOSED ([j, i], source nodes on partitions) so both
the softmax denominators (ones-vector matmuls) and the aggregation are PE
matmuls; adj is PE-transposed on device; e_dst rows are broadcast across
partitions via a DRAM round-trip. Aggregation and denominators run in bf16
(numerator/denominator rounding errors largely cancel); the logit path is
f32 with f32r for the wide matmuls. Aggregation outputs are packed two
node-chunks per PSUM bank so sample k+1's preamble (emitted interleaved
with sample k's attention phase) always has free PSUM slots.
"""

import numpy as np

import concourse.bass as bass
import concourse.bacc as bacc
import concourse.tile as tile
from concourse import mybir
from concourse.bass_utils import run_bass_kernel_spmd
from concourse.masks import make_identity

F32 = mybir.dt.float32
F32R = mybir.dt.float32r
BF16 = mybir.dt.bfloat16
AF = mybir.ActivationFunctionType
ALU = mybir.AluOpType

P = 128          # partitions
N = 1024         # nodes
D = 256          # input feature dim
H = 4            # heads
F = 64           # per-head dim
HF = H * F       # 256
NCH = N // P     # 8 chunks of nodes
NCORES = 8
BPC = 2          # batch samples per core
NEG_SLOPE = 0.2


def build_nc(num_devices=NCORES, repeat=1):
    nc = bacc.Bacc("TRN2", target_bir_lowering=False, debug=False,
                   num_devices=num_devices)
    x_d = nc.dram_tensor("x", [BPC, N, D], F32, kind="ExternalInput")
    adj_d = nc.dram_tensor("adj", [BPC, N, N], F32, kind="ExternalInput")
    w_d = nc.dram_tensor("W", [D, HF], F32, kind="ExternalInput")
    acat_d = nc.dram_tensor("acat", [HF, 2 * H], F32, kind="ExternalInput")
    wa_d = nc.dram_tensor("wa", [D, 2 * H], F32, kind="ExternalInput")
    bias_d = nc.dram_tensor("bias", [HF], F32, kind="ExternalInput")
    out_d = nc.dram_tensor("out", [BPC, N, HF], F32, kind="ExternalOutput")

    with tile.TileContext(nc) as tc:
        with (
            tc.tile_pool(name="consts", bufs=1) as consts,
            tc.tile_pool(name="xs", bufs=1) as p_xs,
            tc.tile_pool(name="xt", bufs=1) as p_xt,
            tc.tile_pool(name="haug", bufs=2) as p_haug,
            tc.tile_pool(name="erow", bufs=2) as p_erow,
            tc.tile_pool(name="ecol", bufs=2) as p_ecol,
            tc.tile_pool(name="bd", bufs=2) as p_bd,
            tc.tile_pool(name="mask", bufs=2) as p_mask,
            tc.tile_pool(name="adj", bufs=1) as p_adj,
            tc.tile_pool(name="pt", bufs=4) as p_pt,
            tc.tile_pool(name="pm", bufs=6) as p_pm,
            tc.tile_pool(name="ssum", bufs=2) as p_ssum,
            tc.tile_pool(name="ot", bufs=2) as p_ot,
            tc.tile_pool(name="ps", bufs=8, space="PSUM") as p_ps,
            tc.tile_pool(name="dram", bufs=2, space="DRAM") as p_dram,
        ):
            ident = consts.tile([P, P], F32)
            make_identity(nc, ident)
            w_sb = consts.tile([P, 2, HF], F32)
            for dc in range(2):
                nc.sync.dma_start(w_sb[:, dc, :], w_d[dc * P:(dc + 1) * P, :])
            bias_bc = consts.tile([P, HF], F32)
            nc.sync.dma_start(bias_bc[:], bias_d[:].partition_broadcast(P))
            w_sbr = consts.tile([P, 2, HF], F32R)
            nc.vector.tensor_copy(w_sbr[:], w_sb[:])
            wa_sb = consts.tile([P, 2, 2 * H], F32)
            for dc in range(2):
                nc.sync.dma_start(wa_sb[:, dc, :], wa_d[dc * P:(dc + 1) * P, :])
            wa_sbr = consts.tile([P, 2, 2 * H], F32R)
            nc.vector.tensor_copy(wa_sbr[:], wa_sb[:])
            alpha_col = consts.tile([P, 1], F32)
            nc.vector.memset(alpha_col[:], NEG_SLOPE)
            ones_rep = consts.tile([P, 64], BF16)
            nc.vector.memset(ones_rep[:], 1.0)

            def phase_abm(b):
                """Generator: yields after each small chunk so the caller can
                interleave this sample's preamble into the previous sample's
                attention phase. Final yield carries the state tuple."""
                # ---- A: load x, PE-transpose to xT [d, i] ----
                xt_t = p_xt.tile([P, 2, N], F32R, tag="xt", name=f"xt{b}")
                xs_all = p_xs.tile([P, NCH, D], F32, tag="xs", name=f"xs{b}")
                nc.sync.dma_start(
                    xs_all[:], x_d[b].rearrange("(c p) d -> p c d", p=P))
                for icg in range(2):
                    for dc in range(2):
                        psx = p_ps.tile([P, 512], F32, tag="u",
                                        name=f"psx{b}_{icg}{dc}")
                        for ic4 in range(4):
                            ic = icg * 4 + ic4
                            nc.tensor.transpose(psx[:, ic4 * P:(ic4 + 1) * P],
                                                xs_all[:, ic, dc * P:(dc + 1) * P],
                                                ident[:])
                        nc.vector.tensor_copy(
                            xt_t[:, dc, icg * 512:(icg + 1) * 512], psx[:])
                        yield

                # E rows [2H, N] = (W @ acat)^T @ xT
                erow_t = p_erow.tile([P, N], F32, tag="erow", name=f"erow{b}")
                nc.gpsimd.memset(erow_t[:], 0.0)
                for nh in range(2):
                    pe_ = p_ps.tile([P, 512], F32, tag="u", name=f"pse{b}{nh}")
                    for dc in range(2):
                        nc.tensor.matmul(pe_[0:2 * H, :],
                                         wa_sbr[:, dc, :],
                                         xt_t[:, dc, nh * 512:(nh + 1) * 512],
                                         start=(dc == 0), stop=(dc == 1))
                    nc.vector.tensor_copy(erow_t[0:2 * H, nh * 512:(nh + 1) * 512],
                                          pe_[0:2 * H, :])
                    yield

                # e columns: transpose E rows -> [node_part, 2H] per chunk
                ecol_t = p_ecol.tile([P, NCH, 2 * H], F32, tag="ecol",
                                     name=f"ecol{b}")
                for jc2 in range(4):
                    pec = p_ps.tile([P, 2, P], F32, tag="u", name=f"pec{b}{jc2}")
                    for k in range(2):
                        jc = jc2 * 2 + k
                        nc.tensor.transpose(pec[:, k, :],
                                            erow_t[:, jc * P:(jc + 1) * P],
                                            ident[:])
                        nc.vector.tensor_copy(ecol_t[:, jc, :], pec[:, k, 0:2 * H])
                    yield

                # e_dst rows broadcast across partitions via DRAM round-trip
                scr = p_dram.tile([2 * H, N], F32, tag="scr", name=f"scr{b}")
                nc.sync.dma_start(scr[:], erow_t[0:2 * H, :])
                bd_t = p_bd.tile([P, H, N], F32, tag="bd", name=f"bd{b}")
                for h in range(H):
                    nc.sync.dma_start(bd_t[:, h, :],
                                      scr[2 * h + 1, :].partition_broadcast(P))
                yield

                # outer-product path staging: exp rows in bf16, packed so that
                # each head's src/dst rows share a legal base partition
                # (h0@0, h1@32, h2@64 blocks 0/1, h3@64 blocks 2/3)
                uex_b = p_erow.tile([2 * H, N], BF16, tag="uex", name=f"uex{b}")
                nc.scalar.activation(out=uex_b[:], in_=erow_t[0:2 * H, :],
                                     func=AF.Exp, bias=0.0, scale=1.0)
                sex_b = p_erow.tile([2 * H, N], BF16, tag="sex", name=f"sex{b}")
                nc.scalar.activation(out=sex_b[:], in_=erow_t[0:2 * H, :],
                                     func=AF.Exp, bias=0.0, scale=NEG_SLOPE)
                upk_t = p_ecol.tile([P, 4, N], BF16, tag="upk", name=f"upk{b}")
                for h in range(H):
                    base = (0, 32, 64, 64)[h]
                    blk = 2 if h == 3 else 0
                    nc.sync.dma_start(upk_t[base:base + 1, blk, :],
                                      uex_b[2 * h:2 * h + 1, :])
                    nc.sync.dma_start(upk_t[base:base + 1, blk + 1, :],
                                      uex_b[2 * h + 1:2 * h + 2, :])
                scr2 = p_dram.tile([2 * H, N], BF16, tag="scr2", name=f"scr2{b}")
                nc.sync.dma_start(scr2[:], sex_b[:])
                bsd_t = p_bd.tile([P, H, N], BF16, tag="bsd", name=f"bsd{b}")
                for h in range(H):
                    nc.sync.dma_start(bsd_t[:, h, :],
                                      scr2[2 * h + 1, :].partition_broadcast(P))
                vex_t = p_ecol.tile([P, NCH, 2 * H], F32, tag="vex", name=f"vex{b}")
                nc.scalar.activation(out=vex_t[:], in_=ecol_t[:],
                                     func=AF.Exp, bias=0.0, scale=NEG_SLOPE)
                yield

                # ---- h (bf16) for aggregation ----
                haug_t = p_haug.tile([P, NCH, H, F], BF16, tag="haug",
                                     name=f"haug{b}")
                for ic in range(NCH):
                    ph = p_ps.tile([P, HF], F32, tag="u", name=f"psh{b}{ic}")
                    for dc in range(2):
                        nc.tensor.matmul(ph[:],
                                         xt_t[:, dc, ic * P:(ic + 1) * P],
                                         w_sbr[:, dc, :],
                                         start=(dc == 0), stop=(dc == 1))
                    nc.vector.tensor_copy(
                        haug_t[:, ic, :, :],
                        ph.rearrange("p (h f) -> p h f", h=H))
                    if ic % 2 == 1:
                        yield

                # ---- M: transposed edge mask (adj^T > 0.5), bf16 ----
                mask_t = p_mask.tile([P, NCH, N], BF16, tag="mask",
                                     name=f"mask{b}")
                for jcp in range(4):   # pairs of j-chunks; 1 psum bank at a time
                    at_all = p_adj.tile([P, NCH, 2 * P], F32, tag="adj",
                                        name=f"at{b}{jcp}")
                    nc.sync.dma_start(
                        at_all[:],
                        adj_d[b].rearrange("(c p) j -> p c j", p=P)
                        [:, :, jcp * 2 * P:(jcp + 1) * 2 * P])
                    for k in range(2):      # the two j-chunks of this pair
                        jc = jcp * 2 + k
                        for g in range(2):  # source-chunk groups of 4
                            pmx = p_ps.tile([P, 512], F32, tag="u",
                                            name=f"pmx{b}_{jcp}{k}{g}")
                            for ib4 in range(4):
                                ib = g * 4 + ib4
                                nc.tensor.transpose(
                                    pmx[:, ib4 * P:(ib4 + 1) * P],
                                    at_all[:, ib, k * P:(k + 1) * P], ident[:])
                            nc.vector.tensor_scalar(
                                out=mask_t[:, jc, g * 512:(g + 1) * 512],
                                in0=pmx[:],
                                scalar1=0.5, scalar2=None, op0=ALU.is_gt)
                            yield

                yield (haug_t, ecol_t, bd_t, mask_t, upk_t, bsd_t, vex_t)

            def run_abm(b):
                """Run the full preamble for sample b, return state."""
                st = None
                for st in phase_abm(b):
                    pass
                return st

            def phase_de(b, state, interleave=None):
                """Attention + aggregation for sample b; optionally pull one
                chunk of `interleave` (next sample's preamble) per tile."""
                haug_t, ecol_t, bd_t, mask_t, upk_t, bsd_t, vex_t = state
                # aggregation outputs: 2 node-chunks packed per PSUM bank
                pouts = [p_ps.tile([P, 512], F32, tag="u", name=f"po{b}_{i}")
                         for i in range(NCH // 2)]
                # softmax denominators: one bank per i-half; head h occupies
                # rows [32h, 32h+32) (M-replicated rows). h3 is written first
                # as a [64, 512] block at base 64, then h0-2 overwrite 0..95.
                sums = [p_ps.tile([P, 512], F32, tag="u", name=f"sm{b}_{i}")
                        for i in range(2)]

                def sum_slot(h, half, for_write=False):
                    if for_write and h == 3:
                        return sums[half][64:128, :]
                    if for_write:
                        return sums[half][32 * h:32 * (h + 1), :]
                    return sums[half][32 * h:32 * h + 1, :]
                nxt = None
                use_max = interleave is None
                for h in (3, 0, 1, 2):
                    base = (0, 32, 64, 64)[h]
                    blk = 2 if h == 3 else 0
                    for jc in range(NCH):
                        pm = p_pm.tile([P, N], BF16, tag="pm", name=f"pm{b}{h}{jc}")
                        if use_max and jc % 2 == 1:
                            # A = exp(z) as rank-1 outer product on PE;
                            # B = exp(0.2 z); p = mask * max(A, B)
                            for q in range(2):
                                psA = p_ps.tile([P, 512], F32, tag="u",
                                                name=f"pA{b}{h}{jc}{q}")
                                nc.tensor.matmul(
                                    psA[:],
                                    upk_t[base:base + 1, blk,
                                          jc * P:(jc + 1) * P],
                                    upk_t[base:base + 1, blk + 1,
                                          q * 512:(q + 1) * 512],
                                    start=True, stop=True)
                                bs = p_pm.tile([P, 512], BF16, tag="bs",
                                               name=f"bs{b}{h}{jc}{q}", bufs=4)
                                nc.vector.tensor_scalar(
                                    out=bs[:],
                                    in0=bsd_t[:, h, q * 512:(q + 1) * 512],
                                    scalar1=vex_t[:, jc, 2 * h:2 * h + 1],
                                    scalar2=None, op0=ALU.mult)
                                nc.vector.tensor_tensor(
                                    out=pm[:, q * 512:(q + 1) * 512],
                                    in0=bs[:], in1=psA[:], op=ALU.max)
                            nc.gpsimd.tensor_tensor(out=pm[:], in0=pm[:],
                                                    in1=mask_t[:, jc, :],
                                                    op=ALU.mult)
                        else:
                            pt = p_pt.tile([P, N], F32, tag="pt",
                                           name=f"pt{b}{h}{jc}")
                            nc.scalar.activation(
                                out=pt[:], in_=bd_t[:, h, :], func=AF.Prelu,
                                bias=ecol_t[:, jc, 2 * h:2 * h + 1],
                                scale=1.0, alpha=alpha_col[:])
                            pe_b = p_pm.tile([P, N], BF16, tag="peb",
                                             name=f"pe{b}{h}{jc}", bufs=4)
                            nc.scalar.activation(out=pe_b[:], in_=pt[:],
                                                 func=AF.Exp, bias=0.0, scale=1.0)
                            eng = nc.gpsimd if jc % 3 == 2 else nc.vector
                            eng.tensor_tensor(out=pm[:], in0=pe_b[:],
                                              in1=mask_t[:, jc, :],
                                              op=ALU.mult)
                        first = (h == 3 and jc == 0)
                        last = (h == 2 and jc == NCH - 1)
                        for ic in range(NCH):
                            nc.tensor.matmul(
                                pouts[ic // 2][:, (ic % 2) * 256 + h * F:
                                               (ic % 2) * 256 + (h + 1) * F],
                                pm[:, ic * P:(ic + 1) * P],
                                haug_t[:, jc, h, :],
                                start=(first and ic % 2 == 0),
                                stop=(last and ic % 2 == 1))
                        nrep = 64 if h == 3 else 32
                        for half in range(2):
                            nc.tensor.matmul(
                                sum_slot(h, half, for_write=True),
                                ones_rep[:, 0:nrep],
                                pm[:, half * 512:(half + 1) * 512],
                                start=(jc == 0), stop=(jc == NCH - 1),
                                skip_group_check=(h != 3))
                        if interleave is not None:
                            nxt = next(interleave, nxt)

                # ---- denominators -> per-chunk reciprocal columns ----
                ssum_sb = p_ssum.tile([P, N], F32, tag="ssum", name=f"ss{b}")
                nc.gpsimd.memset(ssum_sb[:], 1.0)
                for half in range(2):
                    for h in range(H):
                        nc.vector.tensor_copy(
                            ssum_sb[32 * h:32 * h + 1,
                                    half * 512:(half + 1) * 512],
                            sum_slot(h, half))
                recip_t = p_ssum.tile([P, NCH, H], F32, tag="recip",
                                      name=f"rc{b}")
                for ic2 in range(4):
                    prc = p_ps.tile([P, 2, P], F32, tag="u", name=f"prc{b}{ic2}")
                    for k in range(2):
                        ic = ic2 * 2 + k
                        nc.tensor.transpose(prc[:, k, :],
                                            ssum_sb[:, ic * P:(ic + 1) * P],
                                            ident[:])
                        prcv = prc[:, k, :].rearrange("p (h c) -> p h c", c=32)
                        nc.vector.reciprocal(recip_t[:, ic, :], prcv[:, :, 0])

                # ---- normalize + bias + store ----
                for ic in range(NCH):
                    po = pouts[ic // 2].rearrange(
                        "p (q h f) -> p q h f", q=2, h=H)[:, ic % 2]
                    ot = p_ot.tile([P, HF], F32, tag="ot", name=f"ot{b}{ic}")
                    otv = ot.rearrange("p (h f) -> p h f", h=H)
                    rb = recip_t[:, ic, :].unsqueeze(2).broadcast_to([P, H, F])
                    nc.vector.tensor_tensor(out=otv[:], in0=po[:],
                                            in1=rb, op=ALU.mult)
                    nc.gpsimd.tensor_tensor(out=ot[:], in0=ot[:], in1=bias_bc[:],
                                            op=ALU.add)
                    nc.sync.dma_start(out_d[b, ic * P:(ic + 1) * P, :], ot[:])
                return nxt

            def body():
                st0 = run_abm(0)
                gen1 = phase_abm(1)
                st1 = phase_de(0, st0, interleave=gen1)
                for last in gen1:   # drain any remaining preamble chunks
                    st1 = last
                phase_de(1, st1)

            if repeat == 1:
                body()
            else:
                with tc.For_i(0, repeat, 1):
                    body()

    nc.compile()
    return nc


_NC_CACHE = {}


def _get_nc():
    if "nc" not in _NC_CACHE:
        _NC_CACHE["nc"] = build_nc()
    return _NC_CACHE["nc"]


def _prep_weights(W, a_src, a_dst, bias):
    W2 = np.ascontiguousarray(W.reshape(D, HF).astype(np.float32))
    acat = np.zeros((HF, 2 * H), np.float32)
    for h in range(H):
        acat[h * F:(h + 1) * F, 2 * h] = a_src[h]
        acat[h * F:(h + 1) * F, 2 * h + 1] = a_dst[h]
    wa = np.ascontiguousarray((W2 @ acat).astype(np.float32))
    return W2, acat, wa, np.ascontiguousarray(bias.astype(np.float32))


def kernel(x, adj, W, a_src, a_dst, bias):
    x = np.asarray(x, dtype=np.float32)
    adj = np.asarray(adj, dtype=np.float32)
    W2, acat, wa, biasv = _prep_weights(np.asarray(W), np.asarray(a_src),
                                        np.asarray(a_dst), np.asarray(bias))
    nc = _get_nc()
    in_maps = []
    for c in range(NCORES):
        in_maps.append({
            "x": np.ascontiguousarray(x[c * BPC:(c + 1) * BPC]),
            "adj": np.ascontiguousarray(adj[c * BPC:(c + 1) * BPC]),
            "W": W2, "acat": acat, "wa": wa, "bias": biasv,
        })
    r = run_bass_kernel_spmd(nc, in_maps, core_ids=list(range(NCORES)))
    return np.concatenate([r.results[c]["out"] for c in range(NCORES)], axis=0)
